# revision 1
# baseline (speedup 1.0000x reference)
"""MemoryCrossAttention Trainium2 Bass kernel.

8-core data-parallel over query rows: core c handles batch c//2, row-half
c%2 (2048 rows). K/V come from the 256 memory tokens, computed per core.
All matmuls run as float32r (full PE rate at N>=512, ~1e-4 rel precision).
RMSNorm is computed on-chip via a squares pass + ones-matmul partition
reduction; softmax mask folds into the exp bias (per-partition, scoresT
layout); the softmax denominator is a ones-matmul over probsT with the
reciprocal fused into the attention-output eviction.
"""
from concourse.bass_utils import run_bass_kernel_spmd


from contextlib import ExitStack

import concourse.bass as bass
import concourse.tile as tile
from concourse import mybir

F32 = mybir.dt.float32
F32R = mybir.dt.float32r
BF16 = mybir.dt.bfloat16
P = 128


def build(nc, H, NH, R, M, eps=1e-6, phases=4):
    HD = 128
    assert H == NH * HD
    KT = H // P           # contraction tiles
    LQ = R // 512         # 512-wide l chunks
    NHTP = NH // 2        # head pairs (Q/G/O weight streaming)
    MT = M // P           # memory-token partition tiles (2)
    KH = min(8, NH)       # heads per K-proj psum group
    NKG = NH // KH        # K-proj head groups
    KGW = KH * P          # K-proj weight tile width
    NVC = max(1, (NH * HD) // 512)  # V d-chunks of 512
    scale = HD ** -0.5

    xT = nc.dram_tensor("xT", [H, R], F32R, kind="ExternalInput")
    memT = nc.dram_tensor("memT", [H, M], F32R, kind="ExternalInput")
    maskb = nc.dram_tensor("maskb", [P, MT], F32, kind="ExternalInput")
    wqT = nc.dram_tensor("wqT", [NHTP, KT, P, 256], F32R, kind="ExternalInput")
    wgT = nc.dram_tensor("wgT", [NHTP, KT, P, 256], F32R, kind="ExternalInput")
    woT = nc.dram_tensor("woT", [NHTP, KT, P, 256], F32R, kind="ExternalInput")
    wkT = nc.dram_tensor("wkT", [NKG, KT, P, KGW], F32R, kind="ExternalInput")
    wvT = nc.dram_tensor("wvT", [NVC, KT, P, 512], F32R, kind="ExternalInput")
    outT = nc.dram_tensor("outT", [H, R], F32, kind="ExternalOutput")

    with tile.TileContext(nc) as tc, ExitStack() as ctx:
        dram = ctx.enter_context(tc.tile_pool(name="dram", bufs=1, space="DRAM"))
        qspill = dram.tile([H, R], F32R)
        gspill = dram.tile([H, R], F32)
        aspill = dram.tile([H, R], F32R)
        s_scr = dram.tile([R], F32)
        rd_scr = dram.tile([NH, R], F32)

        const = ctx.enter_context(tc.tile_pool(name="const", bufs=1))
        ones_f32 = const.tile([P, 1], F32)
        nc.vector.memset(ones_f32, 1.0)
        ones_sb = const.tile([P, 1], F32R)
        nc.vector.tensor_copy(ones_sb, ones_f32)
        eps_sb = const.tile([1, 1], F32)
        nc.vector.memset(eps_sb, eps)
        mask_sb = const.tile([P, MT], F32)
        nc.sync.dma_start(out=mask_sb, in_=maskb[:])

        # persistent: K/V stay for phases B-C
        kvpool = ctx.enter_context(tc.tile_pool(name="kv", bufs=1))
        kT_big = kvpool.tile([P, NH, M], F32R)    # [d, h, m]
        vmd_big = kvpool.tile([P, MT, H], F32R)   # [m, mt, d_full]

        with tc.tile_pool(name="x", bufs=1) as xpool:
            x_big = xpool.tile([P, KT, R], F32R)  # xT tiles; becomes xnT in place

            # ================= Phase A: load x, rmsnorm =================
            for kt in range(KT):
                nc.sync.dma_start(out=x_big[:, kt, :],
                                  in_=xT[kt * P:(kt + 1) * P, :])

            with tc.tile_pool(name="x2", bufs=2) as x2p, \
                 tc.tile_pool(name="ssqp", bufs=1, space="PSUM") as ssqp, \
                 tc.tile_pool(name="sp", bufs=1) as sp:
                ssq = [ssqp.tile([1, 512], F32, name=f"ssq{i}") for i in range(LQ)]
                for kt in range(KT):
                    x2 = x2p.tile([P, R], F32R)
                    nc.vector.tensor_mul(x2, x_big[:, kt, :], x_big[:, kt, :])
                    for lq in range(LQ):
                        nc.tensor.matmul(
                            ssq[lq], ones_sb, x2[:, lq * 512:(lq + 1) * 512],
                            start=(kt == 0), stop=(kt == KT - 1))
                s_sb = sp.tile([1, R], F32)
                rs_sb = sp.tile([1, R], F32)
                for lq in range(LQ):
                    nc.scalar.activation(
                        s_sb[:, lq * 512:(lq + 1) * 512], ssq[lq],
                        mybir.ActivationFunctionType.Sqrt,
                        bias=eps_sb, scale=1.0 / H)
                nc.vector.reciprocal(rs_sb, s_sb)
                nc.sync.dma_start(out=s_scr[:], in_=rs_sb[0:1, :])

            with tc.tile_pool(name="sbc", bufs=1) as sbcp:
                s_bc = sbcp.tile([P, R], F32)
                s_ap = s_scr[:]
                nc.sync.dma_start(
                    out=s_bc,
                    in_=bass.AP(tensor=s_ap.tensor, offset=s_ap.offset,
                                ap=[[0, P]] + s_ap.ap))
                for kt in range(KT):
                    nc.vector.tensor_mul(x_big[:, kt, :], x_big[:, kt, :], s_bc)

            # ============= Phase B1: K/V from memory tokens =============
            with tc.tile_pool(name="memp", bufs=1) as memp:
                mem_big = memp.tile([P, KT, M], F32R)
                for kt in range(KT):
                    nc.sync.dma_start(out=mem_big[:, kt, :],
                                      in_=memT[kt * P:(kt + 1) * P, :])

                with tc.tile_pool(name="wkst", bufs=3) as wkst, \
                     tc.tile_pool(name="kps", bufs=1, space="PSUM") as kps:
                    for kg in range(NKG):
                        kpsum = [kps.tile([P, M], F32, name=f"kpsum{i}")
                                 for i in range(KH)]
                        for kt in range(KT):
                            wk_t = wkst.tile([P, KGW], F32R)
                            nc.sync.dma_start(out=wk_t, in_=wkT[kg, kt])
                            for hh in range(KH):
                                nc.tensor.matmul(
                                    kpsum[hh], wk_t[:, hh * P:(hh + 1) * P],
                                    mem_big[:, kt, :],
                                    start=(kt == 0), stop=(kt == KT - 1))
                        for hh in range(KH):
                            nc.vector.tensor_copy(kT_big[:, kg * KH + hh, :],
                                                  kpsum[hh])

                with tc.tile_pool(name="wvst", bufs=3) as wvst, \
                     tc.tile_pool(name="vps", bufs=1, space="PSUM") as vps:
                    for dc in range(NVC):
                        vpsum = [vps.tile([P, 512], F32, name=f"vpsum{i}")
                                 for i in range(MT)]
                        for kt in range(KT):
                            wv_t = wvst.tile([P, 512], F32R)
                            nc.sync.dma_start(out=wv_t, in_=wvT[dc, kt])
                            for mt in range(MT):
                                nc.tensor.matmul(
                                    vpsum[mt],
                                    mem_big[:, kt, mt * P:(mt + 1) * P],
                                    wv_t,
                                    start=(kt == 0), stop=(kt == KT - 1))
                        for mt in range(MT):
                            nc.vector.tensor_copy(
                                vmd_big[:, mt, dc * 512:(dc + 1) * 512], vpsum[mt])

            # ============= Phase B2: Q and gate projections =============
            if phases < 2:
                return
            with tc.tile_pool(name="wqst", bufs=3) as wqst, \
                 tc.tile_pool(name="qps", bufs=1, space="PSUM") as qps, \
                 tc.tile_pool(name="qbuf", bufs=4) as qbufp, \
                 tc.tile_pool(name="gbuf", bufs=4) as gbufp:
                for htp in range(NHTP):
                    qpsum = [[qps.tile([P, 512], F32, name=f"qpsum{i}_{j}")
                              for j in range(LQ)] for i in range(2)]
                    for kt in range(KT):
                        wq_t = wqst.tile([P, 256], F32R)
                        nc.sync.dma_start(out=wq_t, in_=wqT[htp, kt])
                        for h2 in range(2):
                            for lq in range(LQ):
                                nc.tensor.matmul(
                                    qpsum[h2][lq], wq_t[:, h2 * P:(h2 + 1) * P],
                                    x_big[:, kt, lq * 512:(lq + 1) * 512],
                                    start=(kt == 0), stop=(kt == KT - 1))
                    for h2 in range(2):
                        ho = htp * 2 + h2
                        for lq in range(LQ):
                            qb = qbufp.tile([P, 512], F32R, name="qb")
                            nc.scalar.copy(qb, qpsum[h2][lq])
                            nc.sync.dma_start(
                                out=qspill[ho * P:(ho + 1) * P,
                                           lq * 512:(lq + 1) * 512],
                                in_=qb)

                for htp in range(NHTP):
                    gpsum = [[qps.tile([P, 512], F32, name=f"qpsum{i}_{j}")
                              for j in range(LQ)] for i in range(2)]
                    for kt in range(KT):
                        wg_t = wqst.tile([P, 256], F32R)
                        nc.sync.dma_start(out=wg_t, in_=wgT[htp, kt])
                        for h2 in range(2):
                            for lq in range(LQ):
                                nc.tensor.matmul(
                                    gpsum[h2][lq], wg_t[:, h2 * P:(h2 + 1) * P],
                                    x_big[:, kt, lq * 512:(lq + 1) * 512],
                                    start=(kt == 0), stop=(kt == KT - 1))
                    for h2 in range(2):
                        ho = htp * 2 + h2
                        for lq in range(LQ):
                            gb = gbufp.tile([P, 512], F32, name="gb")
                            nc.scalar.activation(
                                gb, gpsum[h2][lq],
                                mybir.ActivationFunctionType.Sigmoid)
                            nc.sync.dma_start(
                                out=gspill[ho * P:(ho + 1) * P,
                                           lq * 512:(lq + 1) * 512],
                                in_=gb)

        # ================= Phase C: attention per head =================
        if phases < 3:
            return
        with tc.tile_pool(name="qh", bufs=2) as qhp, \
             tc.tile_pool(name="probs", bufs=2) as probsp, \
             tc.tile_pool(name="rden", bufs=2) as rdenp, \
             tc.tile_pool(name="asb", bufs=2) as asbp, \
             tc.tile_pool(name="sps", bufs=4, space="PSUM") as sps, \
             tc.tile_pool(name="dps", bufs=2, space="PSUM") as dps, \
             tc.tile_pool(name="aps", bufs=2, space="PSUM") as aps:
            for h in range(NH):
                qh = qhp.tile([P, R], F32R, name="qh")
                nc.sync.dma_start(out=qh, in_=qspill[h * P:(h + 1) * P, :])

                probs = probsp.tile([P, MT, R], F32R, name="probs")
                for mt in range(MT):
                    for lq in range(LQ):
                        spsum = sps.tile([P, 512], F32, name="spsum")
                        nc.tensor.matmul(
                            spsum, kT_big[:, h, mt * P:(mt + 1) * P],
                            qh[:, lq * 512:(lq + 1) * 512],
                            start=True, stop=True)
                        nc.scalar.activation(
                            probs[:, mt, lq * 512:(lq + 1) * 512], spsum,
                            mybir.ActivationFunctionType.Exp,
                            bias=mask_sb[:, mt:mt + 1], scale=scale)

                rden = rdenp.tile([1, R], F32, name="rden")
                for lq in range(LQ):
                    dpsum = dps.tile([1, 512], F32, name="dpsum")
                    for mt in range(MT):
                        nc.tensor.matmul(
                            dpsum, ones_sb,
                            probs[:, mt, lq * 512:(lq + 1) * 512],
                            start=(mt == 0), stop=(mt == MT - 1))
                    nc.vector.reciprocal(rden[:, lq * 512:(lq + 1) * 512], dpsum)
                nc.sync.dma_start(out=rd_scr[h:h + 1, :], in_=rden[0:1, :])

                rden_bc = rdenp.tile([P, R], F32, name="rden_bc")
                rd_ap = rd_scr[h, :]
                nc.sync.dma_start(
                    out=rden_bc,
                    in_=bass.AP(tensor=rd_ap.tensor, offset=rd_ap.offset,
                                ap=[[0, P]] + rd_ap.ap))

                attn_sb = asbp.tile([P, R], F32R, name="attn_sb")
                for lq in range(LQ):
                    apsum = aps.tile([P, 512], F32, name="apsum")
                    for mt in range(MT):
                        nc.tensor.matmul(
                            apsum, vmd_big[:, mt, h * P:(h + 1) * P],
                            probs[:, mt, lq * 512:(lq + 1) * 512],
                            start=(mt == 0), stop=(mt == MT - 1))
                    nc.vector.tensor_mul(
                        attn_sb[:, lq * 512:(lq + 1) * 512], apsum,
                        rden_bc[:, lq * 512:(lq + 1) * 512])
                nc.sync.dma_start(out=aspill[h * P:(h + 1) * P, :], in_=attn_sb)

        # ================= Phase D: O-proj + gate =================
        if phases < 4:
            return
        with tc.tile_pool(name="at", bufs=1) as atp, \
             tc.tile_pool(name="wost", bufs=3) as wost, \
             tc.tile_pool(name="gin", bufs=2) as ginp, \
             tc.tile_pool(name="osb", bufs=2) as osbp, \
             tc.tile_pool(name="ops", bufs=1, space="PSUM") as ops:
            at_big = atp.tile([P, KT, R], F32R)
            for kt in range(KT):
                nc.sync.dma_start(out=at_big[:, kt, :],
                                  in_=aspill[kt * P:(kt + 1) * P, :])
            for htp in range(NHTP):
                opsum = [[ops.tile([P, 512], F32, name=f"opsum{i}_{j}")
                          for j in range(LQ)] for i in range(2)]
                for kt in range(KT):
                    wo_t = wost.tile([P, 256], F32R)
                    nc.sync.dma_start(out=wo_t, in_=woT[htp, kt])
                    for h2 in range(2):
                        for lq in range(LQ):
                            nc.tensor.matmul(
                                opsum[h2][lq], wo_t[:, h2 * P:(h2 + 1) * P],
                                at_big[:, kt, lq * 512:(lq + 1) * 512],
                                start=(kt == 0), stop=(kt == KT - 1))
                for h2 in range(2):
                    ho = htp * 2 + h2
                    g_in = ginp.tile([P, R], F32, name="g_in")
                    nc.sync.dma_start(out=g_in,
                                      in_=gspill[ho * P:(ho + 1) * P, :])
                    o_sb = osbp.tile([P, R], F32, name="o_sb")
                    for lq in range(LQ):
                        nc.vector.tensor_mul(
                            o_sb[:, lq * 512:(lq + 1) * 512], opsum[h2][lq],
                            g_in[:, lq * 512:(lq + 1) * 512])
                    nc.sync.dma_start(out=outT[ho * P:(ho + 1) * P, :], in_=o_sb)

    nc.compile()
    return nc


def prep_inputs(hs_slice, mem_b, mask_b, norm_w, wq, wk, wv, wo, wg, NH):
    """Host-side prep for one core. hs_slice [R, H], mem_b [M, H], mask_b [M]."""
    import numpy as np
    H = hs_slice.shape[1]
    M = mem_b.shape[0]
    P = 128
    KT = H // P
    KH = min(8, NH)
    KGW = KH * P

    def tile_w(wT, width):
        # wT [H, H] -> [H//width, KT, 128, width]
        n = wT.shape[1] // width
        return np.ascontiguousarray(
            wT.reshape(KT, P, n, width).transpose(2, 0, 1, 3))

    wq_n = (wq * norm_w[None, :]).T.astype(np.float32)   # [in, out]
    wg_n = (wg * norm_w[None, :]).T.astype(np.float32)
    wo_t = wo.T.astype(np.float32)
    wk_t = wk.T.astype(np.float32)
    wv_t = wv.T.astype(np.float32)

    maskb = np.where(mask_b, 0.0, -50.0).astype(np.float32)
    maskb = np.ascontiguousarray(maskb.reshape(M // P, P).T)  # [128, MT]

    return {
        "xT": np.ascontiguousarray(hs_slice.T.astype(np.float32)),
        "memT": np.ascontiguousarray(mem_b.T.astype(np.float32)),
        "maskb": maskb,
        "wqT": tile_w(wq_n, 256),
        "wgT": tile_w(wg_n, 256),
        "woT": tile_w(wo_t, 256),
        "wkT": tile_w(wk_t, KGW),
        "wvT": tile_w(wv_t, 512),
    }


import numpy as np

_H, _NH, _HD, _M = 2048, 16, 128, 256
_B, _L = 4, 4096
_RPC = 2048          # rows per core
_NCORES = 8
_EPS = 1e-6

_nc_cache = [None]


def _prep_core(hs_slice, mem_b, mask_b, shared):
    inp = dict(shared)
    inp["xT"] = np.ascontiguousarray(hs_slice.T)
    inp["memT"] = np.ascontiguousarray(mem_b.T)
    maskb = np.where(mask_b, 0.0, -50.0).astype(np.float32)
    inp["maskb"] = np.ascontiguousarray(maskb.reshape(_M // 128, 128).T)
    return inp


def _tile_w(wT, width):
    KT = wT.shape[0] // 128
    n = wT.shape[1] // width
    return np.ascontiguousarray(
        wT.reshape(KT, 128, n, width).transpose(2, 0, 1, 3))


def kernel(hidden_states, memory_tokens, memory_mask, norm_w,
           wq, wk, wv, wo, wg):
    import concourse.bacc as bacc

    hs = np.asarray(hidden_states, dtype=np.float32)
    mem = np.asarray(memory_tokens, dtype=np.float32)
    mask = np.asarray(memory_mask)
    norm_w = np.asarray(norm_w, dtype=np.float32)

    wq_n = (np.asarray(wq, dtype=np.float32) * norm_w[None, :]).T
    wg_n = (np.asarray(wg, dtype=np.float32) * norm_w[None, :]).T
    shared = {
        "wqT": _tile_w(np.ascontiguousarray(wq_n), 256),
        "wgT": _tile_w(np.ascontiguousarray(wg_n), 256),
        "woT": _tile_w(np.ascontiguousarray(np.asarray(wo, dtype=np.float32).T), 256),
        "wkT": _tile_w(np.ascontiguousarray(np.asarray(wk, dtype=np.float32).T), 1024),
        "wvT": _tile_w(np.ascontiguousarray(np.asarray(wv, dtype=np.float32).T), 512),
    }

    in_maps = []
    for c in range(_NCORES):
        b, half = c // 2, c % 2
        hs_slice = hs[b, half * _RPC:(half + 1) * _RPC, :]
        in_maps.append(_prep_core(hs_slice, mem[b], mask[b], shared))

    if _nc_cache[0] is None:
        nc = bacc.Bacc(None, target_bir_lowering=False, debug=False)
        build(nc, _H, _NH, _RPC, _M, eps=_EPS)
        _nc_cache[0] = nc
    nc = _nc_cache[0]

    import os
    trace = os.environ.get("KERNEL_TRACE") == "1"
    res = run_bass_kernel_spmd(nc, in_maps, core_ids=list(range(_NCORES)),
                               trace=trace)
    kernel.last_result = res

    out = np.empty((_B, _L, _H), dtype=np.float32)
    for c in range(_NCORES):
        b, half = c // 2, c % 2
        out[b, half * _RPC:(half + 1) * _RPC, :] = res.results[c]["outT"].T
    return out



# revision 2
# speedup vs baseline: 1.5926x; 1.5926x over previous
"""MemoryCrossAttention Trainium2 Bass kernel (bf16, SBUF-resident).

8-core data-parallel over query rows: core c handles batch c//2, row-half
c%2 (2048 rows).  All GEMMs run in bf16 (fp32 PSUM accumulation); end-to-end
numpy emulation of the bf16 pipeline gives ~6e-3 rel err vs the fp32 oracle.

Differences vs the fp32r baseline:
 - No DRAM spills: Q, gate input x, and attention outputs A live in SBUF
   (bf16); A overwrites Q head-by-head (same [128d, R] shape).
 - RMSNorm: sum-of-squares matmul uses a [128,128] all-ones stationary so
   the row-sum lands REPLICATED across all 128 partitions -> the 1/rms
   factor needs no partition broadcast; it is folded into the Q-eviction
   multiply (and the gate projection's pre-sigmoid scale in phase D).
 - Softmax denominators use the same replicated-ones trick: no per-head
   DRAM round-trip / broadcast DMA; reciprocal runs on DVE in parallel.
 - Gate projection is deferred to phase D and fused with the O-projection
   eviction (sigmoid on ACT, product on DVE), halving phase-B PE work.
"""
from concourse.bass_utils import run_bass_kernel_spmd

from contextlib import ExitStack

import concourse.bass as bass
import concourse.tile as tile
from concourse import mybir

F32 = mybir.dt.float32
BF16 = mybir.dt.bfloat16
P = 128


def build(nc, H, NH, R, M, eps=1e-6):
    HD = 128
    assert H == NH * HD
    KT = H // P           # contraction tiles over H
    LQ = R // 512         # 512-wide query-row chunks
    MT = M // P           # memory-token partition tiles
    KH = min(8, NH)       # heads per K-proj psum group
    NKG = NH // KH
    KGW = KH * P
    NVC = max(1, (NH * HD) // 512)  # V d-chunks of 512
    scale = HD ** -0.5

    xT = nc.dram_tensor("xT", [H, R], BF16, kind="ExternalInput")
    memT = nc.dram_tensor("memT", [H, M], BF16, kind="ExternalInput")
    maskb = nc.dram_tensor("maskb", [P, MT], F32, kind="ExternalInput")
    wqT = nc.dram_tensor("wqT", [NH, P, H], BF16, kind="ExternalInput")
    wgT = nc.dram_tensor("wgT", [NH, P, H], BF16, kind="ExternalInput")
    woT = nc.dram_tensor("woT", [NH, P, H], BF16, kind="ExternalInput")
    wkT = nc.dram_tensor("wkT", [NKG, KT, P, KGW], BF16, kind="ExternalInput")
    wvT = nc.dram_tensor("wvT", [NVC, KT, P, 512], BF16, kind="ExternalInput")
    outT = nc.dram_tensor("outT", [H, R], BF16, kind="ExternalOutput")

    with tile.TileContext(nc) as tc, ExitStack() as ctx:
        const = ctx.enter_context(tc.tile_pool(name="const", bufs=1))
        ones_f32 = const.tile([P, P], F32)
        nc.vector.memset(ones_f32, 1.0)
        ones_bf = const.tile([P, P], BF16)
        nc.vector.tensor_copy(ones_bf, ones_f32)
        eps_sb = const.tile([P, 1], F32)
        nc.vector.memset(eps_sb, eps)
        mask_sb = const.tile([P, MT], F32)
        nc.sync.dma_start(out=mask_sb, in_=maskb[:])

        # persistent SBUF tensors
        big = ctx.enter_context(tc.tile_pool(name="big", bufs=1))
        x_big = big.tile([P, KT, R], BF16)      # xT tiles (unnormalized)
        qa_big = big.tile([P, NH, R], BF16)     # Q, overwritten by A per head
        kT_big = big.tile([P, NH, M], BF16)     # [d, h, m]
        vmd_big = big.tile([P, MT, H], BF16)    # [m, mt, d_full]
        s_bc = big.tile([P, R], F32)            # 1/rms, replicated partitions

        # x loads issued first: longest dependency chain
        for kt in range(KT):
            nc.sync.dma_start(out=x_big[:, kt, :],
                              in_=xT[kt * P:(kt + 1) * P, :])

        # ============= Phase B1: K/V from memory tokens =============
        with tc.tile_pool(name="memp", bufs=1) as memp:
            mem_big = memp.tile([P, KT, M], BF16)
            for kt in range(KT):
                nc.sync.dma_start(out=mem_big[:, kt, :],
                                  in_=memT[kt * P:(kt + 1) * P, :])

            with tc.tile_pool(name="wkst", bufs=3) as wkst, \
                 tc.tile_pool(name="kps", bufs=1, space="PSUM") as kps:
                for kg in range(NKG):
                    kpsum = [kps.tile([P, M], F32, name=f"kpsum{i}")
                             for i in range(KH)]
                    for kt in range(KT):
                        wk_t = wkst.tile([P, KGW], BF16)
                        nc.sync.dma_start(out=wk_t, in_=wkT[kg, kt])
                        for hh in range(KH):
                            nc.tensor.matmul(
                                kpsum[hh], wk_t[:, hh * P:(hh + 1) * P],
                                mem_big[:, kt, :],
                                start=(kt == 0), stop=(kt == KT - 1))
                    for hh in range(KH):
                        nc.scalar.copy(kT_big[:, kg * KH + hh, :], kpsum[hh])

            with tc.tile_pool(name="wvst", bufs=3) as wvst, \
                 tc.tile_pool(name="vps", bufs=2, space="PSUM") as vps:
                for dc in range(NVC):
                    vpsum = [vps.tile([P, 512], F32, name=f"vpsum{i}")
                             for i in range(MT)]
                    for kt in range(KT):
                        wv_t = wvst.tile([P, 512], BF16)
                        nc.sync.dma_start(out=wv_t, in_=wvT[dc, kt])
                        for mt in range(MT):
                            nc.tensor.matmul(
                                vpsum[mt],
                                mem_big[:, kt, mt * P:(mt + 1) * P],
                                wv_t,
                                start=(kt == 0), stop=(kt == KT - 1))
                    for mt in range(MT):
                        nc.scalar.copy(
                            vmd_big[:, mt, dc * 512:(dc + 1) * 512], vpsum[mt])

        # ================= Phase A: rmsnorm scale =================
        # ssq matmuls vs a [128,128] ones stationary -> result replicated
        # on every partition; no broadcast needed afterwards.
        with tc.tile_pool(name="x2p", bufs=2) as x2p, \
             tc.tile_pool(name="ssqp", bufs=1, space="PSUM") as ssqp, \
             tc.tile_pool(name="sp", bufs=1) as sp:
            ssq = [ssqp.tile([P, 512], F32, name=f"ssq{i}") for i in range(LQ)]
            for kt in range(KT):
                x2 = x2p.tile([P, R], BF16)
                nc.vector.tensor_mul(x2, x_big[:, kt, :], x_big[:, kt, :])
                for lq in range(LQ):
                    nc.tensor.matmul(
                        ssq[lq], ones_bf, x2[:, lq * 512:(lq + 1) * 512],
                        start=(kt == 0), stop=(kt == KT - 1))
            s_sqrt = sp.tile([P, R], F32)
            for lq in range(LQ):
                nc.scalar.activation(
                    s_sqrt[:, lq * 512:(lq + 1) * 512], ssq[lq],
                    mybir.ActivationFunctionType.Sqrt,
                    bias=eps_sb, scale=1.0 / H)
                nc.vector.reciprocal(s_bc[:, lq * 512:(lq + 1) * 512],
                                     s_sqrt[:, lq * 512:(lq + 1) * 512])

        # ============= Phase B2: Q projection =============
        with tc.tile_pool(name="wqst", bufs=2) as wqst, \
             tc.tile_pool(name="qps", bufs=8, space="PSUM") as qps:
            for ho in range(NH):
                wq_sb = wqst.tile([P, H], BF16, name="wq_sb")
                nc.sync.dma_start(out=wq_sb, in_=wqT[ho])
                qpsum = [qps.tile([P, 512], F32, name="qpsum")
                         for _ in range(LQ)]
                for kt in range(KT):
                    for lq in range(LQ):
                        nc.tensor.matmul(
                            qpsum[lq], wq_sb[:, kt * P:(kt + 1) * P],
                            x_big[:, kt, lq * 512:(lq + 1) * 512],
                            start=(kt == 0), stop=(kt == KT - 1))
                for lq in range(LQ):
                    nc.vector.tensor_mul(
                        qa_big[:, ho, lq * 512:(lq + 1) * 512], qpsum[lq],
                        s_bc[:, lq * 512:(lq + 1) * 512])

        # ================= Phase C: attention per head =================
        with tc.tile_pool(name="probs", bufs=2) as probsp, \
             tc.tile_pool(name="rden", bufs=4) as rdenp, \
             tc.tile_pool(name="sps", bufs=4, space="PSUM") as sps, \
             tc.tile_pool(name="dps", bufs=2, space="PSUM") as dps, \
             tc.tile_pool(name="aps", bufs=2, space="PSUM") as aps:
            for h in range(NH):
                probs = probsp.tile([P, MT, R], BF16, name="probs")
                for lq in range(LQ):
                    for mt in range(MT):
                        spsum = sps.tile([P, 512], F32, name="spsum")
                        nc.tensor.matmul(
                            spsum, kT_big[:, h, mt * P:(mt + 1) * P],
                            qa_big[:, h, lq * 512:(lq + 1) * 512],
                            start=True, stop=True)
                        nc.scalar.activation(
                            probs[:, mt, lq * 512:(lq + 1) * 512], spsum,
                            mybir.ActivationFunctionType.Exp,
                            bias=mask_sb[:, mt:mt + 1], scale=scale)

                rdens = []
                for lq in range(LQ):
                    dpsum = dps.tile([P, 512], F32, name="dpsum")
                    for mt in range(MT):
                        nc.tensor.matmul(
                            dpsum, ones_bf,
                            probs[:, mt, lq * 512:(lq + 1) * 512],
                            start=(mt == 0), stop=(mt == MT - 1))
                    rden = rdenp.tile([P, 512], F32, name="rden")
                    nc.vector.reciprocal(rden, dpsum)
                    rdens.append(rden)

                for lq in range(LQ):
                    apsum = aps.tile([P, 512], F32, name="apsum")
                    for mt in range(MT):
                        nc.tensor.matmul(
                            apsum, vmd_big[:, mt, h * P:(h + 1) * P],
                            probs[:, mt, lq * 512:(lq + 1) * 512],
                            start=(mt == 0), stop=(mt == MT - 1))
                    nc.vector.tensor_mul(
                        qa_big[:, h, lq * 512:(lq + 1) * 512], apsum,
                        rdens[lq])

        # ============ Phase D: O-proj + gate-proj, fused evict ============
        with tc.tile_pool(name="wost", bufs=2) as wost, \
             tc.tile_pool(name="gsb", bufs=4) as gsbp, \
             tc.tile_pool(name="osb", bufs=4) as osbp, \
             tc.tile_pool(name="ops", bufs=4, space="PSUM") as ops, \
             tc.tile_pool(name="gps", bufs=4, space="PSUM") as gps:
            for ho in range(NH):
                wo_sb = wost.tile([P, H], BF16, name="wo_sb")
                nc.sync.dma_start(out=wo_sb, in_=woT[ho])
                wg_sb = wost.tile([P, H], BF16, name="wg_sb")
                nc.sync.dma_start(out=wg_sb, in_=wgT[ho])
                for lq in range(LQ):
                    opsum = ops.tile([P, 512], F32, name="opsum")
                    gpsum = gps.tile([P, 512], F32, name="gpsum")
                    for kt in range(KT):
                        nc.tensor.matmul(
                            opsum, wo_sb[:, kt * P:(kt + 1) * P],
                            qa_big[:, kt, lq * 512:(lq + 1) * 512],
                            start=(kt == 0), stop=(kt == KT - 1))
                        nc.tensor.matmul(
                            gpsum, wg_sb[:, kt * P:(kt + 1) * P],
                            x_big[:, kt, lq * 512:(lq + 1) * 512],
                            start=(kt == 0), stop=(kt == KT - 1))
                    # gate = sigmoid(gpsum * 1/rms); out = gate * opsum
                    g_sb = gsbp.tile([P, 512], F32, name="g_sb")
                    nc.vector.tensor_mul(
                        g_sb, gpsum, s_bc[:, lq * 512:(lq + 1) * 512])
                    g_sg = gsbp.tile([P, 512], BF16, name="g_sg")
                    nc.scalar.activation(
                        g_sg, g_sb, mybir.ActivationFunctionType.Sigmoid)
                    o_sb = osbp.tile([P, 512], BF16, name="o_sb")
                    nc.vector.tensor_mul(o_sb, opsum, g_sg)
                    nc.sync.dma_start(
                        out=outT[ho * P:(ho + 1) * P,
                                 lq * 512:(lq + 1) * 512],
                        in_=o_sb)

    nc.compile()
    return nc


import numpy as np
import ml_dtypes

BF_NP = ml_dtypes.bfloat16

_H, _NH, _HD, _M = 2048, 16, 128, 256
_B, _L = 4, 4096
_RPC = 2048          # rows per core
_NCORES = 8
_EPS = 1e-6

_nc_cache = [None]


def _pack_proj(wT):
    """[H, H] lhsT (in, out) -> [NH, 128, H] per-head-column staging:
    arr[ho, p, kt*128 + c] = wT[kt*128 + p, ho*128 + c]."""
    KT = wT.shape[0] // 128
    NH = wT.shape[1] // 128
    t = wT.reshape(KT, 128, NH, 128)            # kt, p, ho, c
    return np.ascontiguousarray(t.transpose(2, 1, 0, 3).reshape(NH, 128, KT * 128))


def _tile_w(wT, width):
    KT = wT.shape[0] // 128
    n = wT.shape[1] // width
    return np.ascontiguousarray(
        wT.reshape(KT, 128, n, width).transpose(2, 0, 1, 3))


def kernel(hidden_states, memory_tokens, memory_mask, norm_w,
           wq, wk, wv, wo, wg):
    import concourse.bacc as bacc

    hs = np.asarray(hidden_states, dtype=np.float32)
    mem = np.asarray(memory_tokens, dtype=np.float32)
    mask = np.asarray(memory_mask)
    norm_w = np.asarray(norm_w, dtype=np.float32)

    wq_n = (np.asarray(wq, dtype=np.float32) * norm_w[None, :]).T
    wg_n = (np.asarray(wg, dtype=np.float32) * norm_w[None, :]).T
    shared = {
        "wqT": _pack_proj(wq_n).astype(BF_NP),
        "wgT": _pack_proj(wg_n).astype(BF_NP),
        "woT": _pack_proj(np.asarray(wo, dtype=np.float32).T).astype(BF_NP),
        "wkT": _tile_w(np.asarray(wk, dtype=np.float32).T, 1024).astype(BF_NP),
        "wvT": _tile_w(np.asarray(wv, dtype=np.float32).T, 512).astype(BF_NP),
    }

    in_maps = []
    for c in range(_NCORES):
        b, half = c // 2, c % 2
        inp = dict(shared)
        inp["xT"] = np.ascontiguousarray(
            hs[b, half * _RPC:(half + 1) * _RPC, :].T).astype(BF_NP)
        inp["memT"] = np.ascontiguousarray(mem[b].T).astype(BF_NP)
        maskb = np.where(mask[b], 0.0, -50.0).astype(np.float32)
        inp["maskb"] = np.ascontiguousarray(maskb.reshape(_M // 128, 128).T)
        in_maps.append(inp)

    if _nc_cache[0] is None:
        nc = bacc.Bacc(None, target_bir_lowering=False, debug=False)
        build(nc, _H, _NH, _RPC, _M, eps=_EPS)
        _nc_cache[0] = nc
    nc = _nc_cache[0]

    import os
    trace = os.environ.get("KERNEL_TRACE") == "1"
    res = run_bass_kernel_spmd(nc, in_maps, core_ids=list(range(_NCORES)),
                               trace=trace)
    kernel.last_result = res

    out = np.empty((_B, _L, _H), dtype=np.float32)
    for c in range(_NCORES):
        b, half = c // 2, c % 2
        out[b, half * _RPC:(half + 1) * _RPC, :] = \
            res.results[c]["outT"].T.astype(np.float32)
    return out


# revision 7
# speedup vs baseline: 1.8050x; 1.1333x over previous
"""MemoryCrossAttention Trainium2 Bass kernel (bf16, SBUF-resident).

8-core data-parallel over query rows: core c handles batch c//2, row-half
c%2 (2048 rows).  All GEMMs run in bf16 (fp32 PSUM accumulation); end-to-end
numpy emulation of the bf16 pipeline gives ~6e-3 rel err vs the fp32 oracle.

Structure:
 - No DRAM spills: x, Q, and attention outputs A live in SBUF (bf16);
   A overwrites Q head-by-head (same [128d, R] shape).
 - RMSNorm: sum-of-squares matmul uses a [128,128] all-ones stationary so
   the row-sum lands REPLICATED across all 128 partitions; x is scaled by
   1/rms in place (also feeds the phase-D gate projection pre-scaled).
 - Softmax denominators use the same replicated-ones trick (no broadcast),
   with single-op approximate DVE reciprocals (~18 bits, 5x faster).
 - Gate projection is deferred to phase D and fused with the O-projection
   eviction (sigmoid straight off PSUM on ACT, product on DVE).
   (Matmul moving width is capped at 512 fp32 psum columns by the ISA.)
"""
from concourse.bass_utils import run_bass_kernel_spmd

from contextlib import ExitStack

import concourse.bass as bass
import concourse.tile as tile
from concourse import mybir

F32 = mybir.dt.float32
BF16 = mybir.dt.bfloat16
P = 128


def build(nc, H, NH, R, M, eps=1e-6):
    HD = 128
    assert H == NH * HD
    KT = H // P           # contraction tiles over H
    LQ = R // 512         # 512-wide query-row chunks (phase C)
    LH = R // 1024        # 1024-wide chunks (ssq/Q/D)
    MT = M // P           # memory-token partition tiles
    KH = min(8, NH)       # heads per K-proj psum group
    NKG = NH // KH
    KGW = KH * P
    NVC = max(1, (NH * HD) // 512)  # V d-chunks of 512
    scale = HD ** -0.5

    xT = nc.dram_tensor("xT", [H, R], BF16, kind="ExternalInput")
    memT = nc.dram_tensor("memT", [H, M], BF16, kind="ExternalInput")
    maskb = nc.dram_tensor("maskb", [P, MT], F32, kind="ExternalInput")
    wqT = nc.dram_tensor("wqT", [NH, P, H], BF16, kind="ExternalInput")
    wgT = nc.dram_tensor("wgT", [NH, P, H], BF16, kind="ExternalInput")
    woT = nc.dram_tensor("woT", [NH, P, H], BF16, kind="ExternalInput")
    wkT = nc.dram_tensor("wkT", [NKG, KT, P, KGW], BF16, kind="ExternalInput")
    wvT = nc.dram_tensor("wvT", [NVC, KT, P, 512], BF16, kind="ExternalInput")
    outT = nc.dram_tensor("outT", [H, R], BF16, kind="ExternalOutput")

    with tile.TileContext(nc) as tc, ExitStack() as ctx:
        const = ctx.enter_context(tc.tile_pool(name="const", bufs=1))
        ones_f32 = const.tile([P, P], F32)
        nc.vector.memset(ones_f32, 1.0)
        ones_bf = const.tile([P, P], BF16)
        nc.vector.tensor_copy(ones_bf, ones_f32)
        eps_sb = const.tile([P, 1], F32)
        nc.vector.memset(eps_sb, eps)
        mask_sb = const.tile([P, MT], F32)
        nc.sync.dma_start(out=mask_sb, in_=maskb[:])

        # persistent SBUF tensors
        big = ctx.enter_context(tc.tile_pool(name="big", bufs=1))
        x_big = big.tile([P, KT, R], BF16)      # xT tiles; scaled in place
        qa_big = big.tile([P, NH, R], BF16)     # Q, overwritten by A per head
        kT_big = big.tile([P, NH, M], BF16)     # [d, h, m]
        vmd_big = big.tile([P, MT, H], BF16)    # [m, mt, d_full]
        s_bc = big.tile([P, R], F32)            # 1/rms, replicated partitions

        # ============= Phase B1: K/V from memory tokens =============
        with tc.tile_pool(name="memp", bufs=1) as memp:
            mem_big = memp.tile([P, KT, M], BF16)
            for kt in range(KT):
                nc.sync.dma_start(out=mem_big[:, kt, :],
                                  in_=memT[kt * P:(kt + 1) * P, :])

            with tc.tile_pool(name="wkst", bufs=3) as wkst, \
                 tc.tile_pool(name="kps", bufs=1, space="PSUM") as kps:
                for kg in range(NKG):
                    kpsum = [kps.tile([P, M], F32, name=f"kpsum{i}")
                             for i in range(KH)]
                    for kt in range(KT):
                        wk_t = wkst.tile([P, KGW], BF16)
                        nc.sync.dma_start(out=wk_t, in_=wkT[kg, kt])
                        for hh in range(KH):
                            nc.tensor.matmul(
                                kpsum[hh], wk_t[:, hh * P:(hh + 1) * P],
                                mem_big[:, kt, :],
                                start=(kt == 0), stop=(kt == KT - 1))
                    for hh in range(KH):
                        nc.scalar.copy(kT_big[:, kg * KH + hh, :], kpsum[hh])

            # x loads: needed by ssq (below); emitted after the K-proj
            # streams so the K weights aren't head-of-line blocked.
            for kt in range(KT):
                nc.sync.dma_start(out=x_big[:, kt, :],
                                  in_=xT[kt * P:(kt + 1) * P, :])

            with tc.tile_pool(name="wvst", bufs=3) as wvst, \
                 tc.tile_pool(name="vps", bufs=2, space="PSUM") as vps:
                for dc in range(NVC):
                    vpsum = [vps.tile([P, 512], F32, name=f"vpsum{i}")
                             for i in range(MT)]
                    for kt in range(KT):
                        wv_t = wvst.tile([P, 512], BF16)
                        nc.sync.dma_start(out=wv_t, in_=wvT[dc, kt])
                        for mt in range(MT):
                            nc.tensor.matmul(
                                vpsum[mt],
                                mem_big[:, kt, mt * P:(mt + 1) * P],
                                wv_t,
                                start=(kt == 0), stop=(kt == KT - 1))
                    for mt in range(MT):
                        nc.scalar.copy(
                            vmd_big[:, mt, dc * 512:(dc + 1) * 512], vpsum[mt])

        # ================= Phase A: rmsnorm, x scaled in place ============
        # ssq matmuls vs a [128,128] ones stationary -> result replicated
        # on every partition; no broadcast needed afterwards.
        with tc.tile_pool(name="x2p", bufs=2) as x2p, \
             tc.tile_pool(name="ssqp", bufs=1, space="PSUM") as ssqp, \
             tc.tile_pool(name="sp", bufs=1) as sp:
            ssq = [ssqp.tile([P, 512], F32, name=f"ssq{i}") for i in range(LQ)]
            for kt in range(KT):
                x2 = x2p.tile([P, R], BF16)
                nc.vector.tensor_mul(x2, x_big[:, kt, :], x_big[:, kt, :])
                for lq in range(LQ):
                    nc.tensor.matmul(
                        ssq[lq], ones_bf, x2[:, lq * 512:(lq + 1) * 512],
                        start=(kt == 0), stop=(kt == KT - 1))
            s_sqrt = sp.tile([P, R], F32)
            for lq in range(LQ):
                nc.scalar.activation(
                    s_sqrt[:, lq * 512:(lq + 1) * 512], ssq[lq],
                    mybir.ActivationFunctionType.Sqrt,
                    bias=eps_sb, scale=1.0 / H)
                nc.vector.reciprocal_approx_fast(
                    out=s_bc[:, lq * 512:(lq + 1) * 512],
                    in_=s_sqrt[:, lq * 512:(lq + 1) * 512])
            for kt in range(KT):
                nc.vector.tensor_mul(x_big[:, kt, :], x_big[:, kt, :], s_bc)

        # ============= Phase B2: Q projection =============
        with tc.tile_pool(name="wqst", bufs=2) as wqst, \
             tc.tile_pool(name="qps", bufs=8, space="PSUM") as qps:
            for ho in range(NH):
                wq_sb = wqst.tile([P, H], BF16, name="wq_sb")
                nc.sync.dma_start(out=wq_sb, in_=wqT[ho])
                qpsum = [qps.tile([P, 512], F32, name="qpsum")
                         for _ in range(LQ)]
                for kt in range(KT):
                    for lq in range(LQ):
                        nc.tensor.matmul(
                            qpsum[lq], wq_sb[:, kt * P:(kt + 1) * P],
                            x_big[:, kt, lq * 512:(lq + 1) * 512],
                            start=(kt == 0), stop=(kt == KT - 1))
                for lq in range(LQ):
                    nc.scalar.copy(
                        qa_big[:, ho, lq * 512:(lq + 1) * 512], qpsum[lq])

        # ================= Phase C: attention per head =================
        with tc.tile_pool(name="probs", bufs=2) as probsp, \
             tc.tile_pool(name="rden", bufs=4) as rdenp, \
             tc.tile_pool(name="sps", bufs=4, space="PSUM") as sps, \
             tc.tile_pool(name="dps", bufs=2, space="PSUM") as dps, \
             tc.tile_pool(name="aps", bufs=2, space="PSUM") as aps:
            for h in range(NH):
                probs = probsp.tile([P, MT, R], BF16, name="probs")
                for lq in range(LQ):
                    for mt in range(MT):
                        spsum = sps.tile([P, 512], F32, name="spsum")
                        nc.tensor.matmul(
                            spsum, kT_big[:, h, mt * P:(mt + 1) * P],
                            qa_big[:, h, lq * 512:(lq + 1) * 512],
                            start=True, stop=True)
                        nc.scalar.activation(
                            probs[:, mt, lq * 512:(lq + 1) * 512], spsum,
                            mybir.ActivationFunctionType.Exp,
                            bias=mask_sb[:, mt:mt + 1], scale=scale)

                rdens = []
                for lq in range(LQ):
                    dpsum = dps.tile([P, 512], F32, name="dpsum")
                    for mt in range(MT):
                        nc.tensor.matmul(
                            dpsum, ones_bf,
                            probs[:, mt, lq * 512:(lq + 1) * 512],
                            start=(mt == 0), stop=(mt == MT - 1))
                    rden = rdenp.tile([P, 512], F32, name="rden")
                    nc.vector.reciprocal_approx_fast(out=rden, in_=dpsum)
                    rdens.append(rden)

                for lq in range(LQ):
                    apsum = aps.tile([P, 512], F32, name="apsum")
                    for mt in range(MT):
                        nc.tensor.matmul(
                            apsum, vmd_big[:, mt, h * P:(h + 1) * P],
                            probs[:, mt, lq * 512:(lq + 1) * 512],
                            start=(mt == 0), stop=(mt == MT - 1))
                    nc.vector.tensor_mul(
                        qa_big[:, h, lq * 512:(lq + 1) * 512], apsum,
                        rdens[lq])

        # ============ Phase D: O-proj + gate-proj, fused evict ============
        with tc.tile_pool(name="wost", bufs=2) as wost, \
             tc.tile_pool(name="gsb", bufs=4) as gsbp, \
             tc.tile_pool(name="osb", bufs=4) as osbp, \
             tc.tile_pool(name="ops", bufs=4, space="PSUM") as ops, \
             tc.tile_pool(name="gps", bufs=4, space="PSUM") as gps:
            for ho in range(NH):
                wo_sb = wost.tile([P, H], BF16, name="wo_sb")
                nc.sync.dma_start(out=wo_sb, in_=woT[ho])
                wg_sb = wost.tile([P, H], BF16, name="wg_sb")
                nc.sync.dma_start(out=wg_sb, in_=wgT[ho])
                for lq in range(LQ):
                    opsum = ops.tile([P, 512], F32, name="opsum")
                    gpsum = gps.tile([P, 512], F32, name="gpsum")
                    for kt in range(KT):
                        nc.tensor.matmul(
                            opsum, wo_sb[:, kt * P:(kt + 1) * P],
                            qa_big[:, kt, lq * 512:(lq + 1) * 512],
                            start=(kt == 0), stop=(kt == KT - 1))
                        nc.tensor.matmul(
                            gpsum, wg_sb[:, kt * P:(kt + 1) * P],
                            x_big[:, kt, lq * 512:(lq + 1) * 512],
                            start=(kt == 0), stop=(kt == KT - 1))
                    # x was pre-scaled by 1/rms -> sigmoid straight off PSUM
                    g_sg = gsbp.tile([P, 512], BF16, name="g_sg")
                    nc.scalar.activation(
                        g_sg, gpsum, mybir.ActivationFunctionType.Sigmoid)
                    o_sb = osbp.tile([P, 512], BF16, name="o_sb")
                    nc.vector.tensor_mul(o_sb, opsum, g_sg)
                    nc.sync.dma_start(
                        out=outT[ho * P:(ho + 1) * P,
                                 lq * 512:(lq + 1) * 512],
                        in_=o_sb)

    nc.compile()
    return nc


import numpy as np
import ml_dtypes

BF_NP = ml_dtypes.bfloat16

_H, _NH, _HD, _M = 2048, 16, 128, 256
_B, _L = 4, 4096
_RPC = 2048          # rows per core
_NCORES = 8
_EPS = 1e-6

_nc_cache = [None]


def _pack_proj(wT):
    """[H, H] lhsT (in, out) -> [NH, 128, H] per-head-column staging:
    arr[ho, p, kt*128 + c] = wT[kt*128 + p, ho*128 + c]."""
    KT = wT.shape[0] // 128
    NH = wT.shape[1] // 128
    t = wT.reshape(KT, 128, NH, 128)            # kt, p, ho, c
    return np.ascontiguousarray(t.transpose(2, 1, 0, 3).reshape(NH, 128, KT * 128))


def _tile_w(wT, width):
    KT = wT.shape[0] // 128
    n = wT.shape[1] // width
    return np.ascontiguousarray(
        wT.reshape(KT, 128, n, width).transpose(2, 0, 1, 3))


def kernel(hidden_states, memory_tokens, memory_mask, norm_w,
           wq, wk, wv, wo, wg):
    import concourse.bacc as bacc

    hs = np.asarray(hidden_states, dtype=np.float32)
    mem = np.asarray(memory_tokens, dtype=np.float32)
    mask = np.asarray(memory_mask)
    norm_w = np.asarray(norm_w, dtype=np.float32)

    wq_n = (np.asarray(wq, dtype=np.float32) * norm_w[None, :]).T
    wg_n = (np.asarray(wg, dtype=np.float32) * norm_w[None, :]).T
    shared = {
        "wqT": _pack_proj(wq_n).astype(BF_NP),
        "wgT": _pack_proj(wg_n).astype(BF_NP),
        "woT": _pack_proj(np.asarray(wo, dtype=np.float32).T).astype(BF_NP),
        "wkT": _tile_w(np.asarray(wk, dtype=np.float32).T, 1024).astype(BF_NP),
        "wvT": _tile_w(np.asarray(wv, dtype=np.float32).T, 512).astype(BF_NP),
    }

    in_maps = []
    for c in range(_NCORES):
        b, half = c // 2, c % 2
        inp = dict(shared)
        inp["xT"] = np.ascontiguousarray(
            hs[b, half * _RPC:(half + 1) * _RPC, :].T).astype(BF_NP)
        inp["memT"] = np.ascontiguousarray(mem[b].T).astype(BF_NP)
        maskb = np.where(mask[b], 0.0, -50.0).astype(np.float32)
        inp["maskb"] = np.ascontiguousarray(maskb.reshape(_M // 128, 128).T)
        in_maps.append(inp)

    if _nc_cache[0] is None:
        nc = bacc.Bacc(None, target_bir_lowering=False, debug=False)
        build(nc, _H, _NH, _RPC, _M, eps=_EPS)
        _nc_cache[0] = nc
    nc = _nc_cache[0]

    import os
    trace = os.environ.get("KERNEL_TRACE") == "1"
    res = run_bass_kernel_spmd(nc, in_maps, core_ids=list(range(_NCORES)),
                               trace=trace)
    kernel.last_result = res

    out = np.empty((_B, _L, _H), dtype=np.float32)
    for c in range(_NCORES):
        b, half = c // 2, c % 2
        out[b, half * _RPC:(half + 1) * _RPC, :] = \
            res.results[c]["outT"].T.astype(np.float32)
    return out


# revision 10
# speedup vs baseline: 1.9639x; 1.0881x over previous
"""MemoryCrossAttention Trainium2 Bass kernel (bf16, SBUF-resident).

8-core data-parallel over query rows: core c handles batch c//2, row-half
c%2 (2048 rows).  All GEMMs run in bf16 (fp32 PSUM accumulation); end-to-end
numpy emulation of the bf16 pipeline gives ~6e-3 rel err vs the fp32 oracle.

Structure:
 - No DRAM spills: x, Q, and attention outputs A live in SBUF (bf16);
   A overwrites Q head-by-head (same [128d, R] shape).
 - RMSNorm: sum-of-squares matmul uses a [128,128] all-ones stationary so
   the row-sum lands REPLICATED across all 128 partitions; the 1/rms factor
   is folded into the Q-proj and gate-proj PSUM evictions (x stays raw, so
   Q matmuls start as soon as x tiles land -- no prescale barrier).
 - V-projection runs after Q so its weights stream during Q's compute;
   K runs first to cover the x input DMA.
 - Softmax denominators use the same replicated-ones trick (no broadcast),
   with single-op approximate DVE reciprocals (~18 bits, 5x faster).
 - Gate projection is deferred to phase D and fused with the O-projection
   eviction (sigmoid straight off PSUM on ACT, product on DVE).
   (Matmul moving width is capped at 512 fp32 psum columns by the ISA.)
"""
from concourse.bass_utils import run_bass_kernel_spmd

from contextlib import ExitStack

import concourse.bass as bass
import concourse.tile as tile
from concourse import mybir

F32 = mybir.dt.float32
BF16 = mybir.dt.bfloat16
P = 128


def build(nc, H, NH, R, M, eps=1e-6):
    HD = 128
    assert H == NH * HD
    KT = H // P           # contraction tiles over H
    LQ = R // 512         # 512-wide query-row chunks (phase C)
    LH = R // 1024        # 1024-wide chunks (ssq/Q/D)
    MT = M // P           # memory-token partition tiles
    KH = min(8, NH)       # heads per K-proj psum group
    NKG = NH // KH
    KGW = KH * P
    NVC = max(1, (NH * HD) // 512)  # V d-chunks of 512
    scale = HD ** -0.5

    xT = nc.dram_tensor("xT", [H, R], BF16, kind="ExternalInput")
    memT = nc.dram_tensor("memT", [H, M], BF16, kind="ExternalInput")
    maskb = nc.dram_tensor("maskb", [P, MT], F32, kind="ExternalInput")
    wqT = nc.dram_tensor("wqT", [NH, P, H], BF16, kind="ExternalInput")
    wgT = nc.dram_tensor("wgT", [NH, P, H], BF16, kind="ExternalInput")
    woT = nc.dram_tensor("woT", [NH, P, H], BF16, kind="ExternalInput")
    wkT = nc.dram_tensor("wkT", [NKG, KT, P, KGW], BF16, kind="ExternalInput")
    wvT = nc.dram_tensor("wvT", [NVC, KT, P, 512], BF16, kind="ExternalInput")
    outT = nc.dram_tensor("outT", [H, R], BF16, kind="ExternalOutput")

    with tile.TileContext(nc) as tc, ExitStack() as ctx:
        const = ctx.enter_context(tc.tile_pool(name="const", bufs=1))
        ones_f32 = const.tile([P, P], F32)
        nc.vector.memset(ones_f32, 1.0)
        ones_bf = const.tile([P, P], BF16)
        nc.vector.tensor_copy(ones_bf, ones_f32)
        eps_sb = const.tile([P, 1], F32)
        nc.vector.memset(eps_sb, eps)
        mask_sb = const.tile([P, MT], F32)
        nc.sync.dma_start(out=mask_sb, in_=maskb[:])

        # persistent SBUF tensors
        big = ctx.enter_context(tc.tile_pool(name="big", bufs=1))
        x_big = big.tile([P, KT, R], BF16)      # xT tiles; scaled in place
        qa_big = big.tile([P, NH, R], BF16)     # Q, overwritten by A per head
        kT_big = big.tile([P, NH, M], BF16)     # [d, h, m]
        vmd_big = big.tile([P, MT, H], BF16)    # [m, mt, d_full]
        s_bc = big.tile([P, R], F32)            # 1/rms, replicated partitions

        # ============= Phase B1a: K projection (covers x input DMA) =======
        memp = ctx.enter_context(tc.tile_pool(name="memp", bufs=1))
        mem_big = memp.tile([P, KT, M], BF16)
        for kt in range(KT):
            nc.sync.dma_start(out=mem_big[:, kt, :],
                              in_=memT[kt * P:(kt + 1) * P, :])

        with tc.tile_pool(name="wkst", bufs=6) as wkst, \
             tc.tile_pool(name="kps", bufs=1, space="PSUM") as kps:
            for kg in range(NKG):
                kpsum = [kps.tile([P, M], F32, name=f"kpsum{i}")
                         for i in range(KH)]
                for kt in range(KT):
                    wk_t = wkst.tile([P, KGW], BF16)
                    nc.sync.dma_start(out=wk_t, in_=wkT[kg, kt])
                    for hh in range(KH):
                        nc.tensor.matmul(
                            kpsum[hh], wk_t[:, hh * P:(hh + 1) * P],
                            mem_big[:, kt, :],
                            start=(kt == 0), stop=(kt == KT - 1))
                for hh in range(KH):
                    nc.scalar.copy(kT_big[:, kg * KH + hh, :], kpsum[hh])

        # x loads: emitted after the K-proj streams so the K weights
        # aren't head-of-line blocked; consumed by ssq/Q below.
        for kt in range(KT):
            nc.sync.dma_start(out=x_big[:, kt, :],
                              in_=xT[kt * P:(kt + 1) * P, :])

        # ================= Phase A: rmsnorm scale =================
        # ssq matmuls vs a [128,128] ones stationary -> result replicated
        # on every partition; no broadcast needed afterwards.  x**2 runs on
        # the ACT engine to keep DVE free for evictions.
        with tc.tile_pool(name="x2p", bufs=2) as x2p, \
             tc.tile_pool(name="ssqp", bufs=1, space="PSUM") as ssqp, \
             tc.tile_pool(name="sp", bufs=1) as sp:
            ssq = [ssqp.tile([P, 512], F32, name=f"ssq{i}") for i in range(LQ)]
            for kt in range(KT):
                x2 = x2p.tile([P, R], BF16)
                nc.scalar.square(x2, x_big[:, kt, :])
                for lq in range(LQ):
                    nc.tensor.matmul(
                        ssq[lq], ones_bf, x2[:, lq * 512:(lq + 1) * 512],
                        start=(kt == 0), stop=(kt == KT - 1))
            s_sqrt = sp.tile([P, R], F32)
            for lq in range(LQ):
                nc.scalar.activation(
                    s_sqrt[:, lq * 512:(lq + 1) * 512], ssq[lq],
                    mybir.ActivationFunctionType.Sqrt,
                    bias=eps_sb, scale=1.0 / H)
                nc.vector.reciprocal_approx_fast(
                    out=s_bc[:, lq * 512:(lq + 1) * 512],
                    in_=s_sqrt[:, lq * 512:(lq + 1) * 512])

        # ============= Phase B2: Q projection (1/rms at eviction) =========
        with tc.tile_pool(name="wqst", bufs=2) as wqst, \
             tc.tile_pool(name="qps", bufs=8, space="PSUM") as qps:
            for ho in range(NH):
                wq_sb = wqst.tile([P, H], BF16, name="wq_sb")
                nc.sync.dma_start(out=wq_sb, in_=wqT[ho])
                qpsum = [qps.tile([P, 512], F32, name="qpsum")
                         for _ in range(LQ)]
                for kt in range(KT):
                    for lq in range(LQ):
                        nc.tensor.matmul(
                            qpsum[lq], wq_sb[:, kt * P:(kt + 1) * P],
                            x_big[:, kt, lq * 512:(lq + 1) * 512],
                            start=(kt == 0), stop=(kt == KT - 1))
                for lq in range(LQ):
                    nc.vector.tensor_mul(
                        qa_big[:, ho, lq * 512:(lq + 1) * 512], qpsum[lq],
                        s_bc[:, lq * 512:(lq + 1) * 512])

        # ===== Phase B1b: V projection (weights streamed during Q) =======
        with tc.tile_pool(name="wvst", bufs=8) as wvst, \
             tc.tile_pool(name="vps", bufs=2, space="PSUM") as vps:
            for dc in range(NVC):
                vpsum = [vps.tile([P, 512], F32, name=f"vpsum{i}")
                         for i in range(MT)]
                for kt in range(KT):
                    wv_t = wvst.tile([P, 512], BF16)
                    nc.sync.dma_start(out=wv_t, in_=wvT[dc, kt])
                    for mt in range(MT):
                        nc.tensor.matmul(
                            vpsum[mt],
                            mem_big[:, kt, mt * P:(mt + 1) * P],
                            wv_t,
                            start=(kt == 0), stop=(kt == KT - 1))
                for mt in range(MT):
                    nc.scalar.copy(
                        vmd_big[:, mt, dc * 512:(dc + 1) * 512], vpsum[mt])

        # ================= Phase C: attention per head =================
        with tc.tile_pool(name="probs", bufs=2) as probsp, \
             tc.tile_pool(name="rden", bufs=4) as rdenp, \
             tc.tile_pool(name="sps", bufs=4, space="PSUM") as sps, \
             tc.tile_pool(name="dps", bufs=2, space="PSUM") as dps, \
             tc.tile_pool(name="aps", bufs=2, space="PSUM") as aps:
            for h in range(NH):
                probs = probsp.tile([P, MT, R], BF16, name="probs")
                for lq in range(LQ):
                    for mt in range(MT):
                        spsum = sps.tile([P, 512], F32, name="spsum")
                        nc.tensor.matmul(
                            spsum, kT_big[:, h, mt * P:(mt + 1) * P],
                            qa_big[:, h, lq * 512:(lq + 1) * 512],
                            start=True, stop=True)
                        nc.scalar.activation(
                            probs[:, mt, lq * 512:(lq + 1) * 512], spsum,
                            mybir.ActivationFunctionType.Exp,
                            bias=mask_sb[:, mt:mt + 1], scale=scale)

                rdens = []
                for lq in range(LQ):
                    dpsum = dps.tile([P, 512], F32, name="dpsum")
                    for mt in range(MT):
                        nc.tensor.matmul(
                            dpsum, ones_bf,
                            probs[:, mt, lq * 512:(lq + 1) * 512],
                            start=(mt == 0), stop=(mt == MT - 1))
                    rden = rdenp.tile([P, 512], F32, name="rden")
                    nc.vector.reciprocal_approx_fast(out=rden, in_=dpsum)
                    rdens.append(rden)

                for lq in range(LQ):
                    apsum = aps.tile([P, 512], F32, name="apsum")
                    for mt in range(MT):
                        nc.tensor.matmul(
                            apsum, vmd_big[:, mt, h * P:(h + 1) * P],
                            probs[:, mt, lq * 512:(lq + 1) * 512],
                            start=(mt == 0), stop=(mt == MT - 1))
                    nc.vector.tensor_mul(
                        qa_big[:, h, lq * 512:(lq + 1) * 512], apsum,
                        rdens[lq])

        # ============ Phase D: O-proj + gate-proj, fused evict ============
        with tc.tile_pool(name="wost", bufs=2) as wost, \
             tc.tile_pool(name="gsb", bufs=4) as gsbp, \
             tc.tile_pool(name="osb", bufs=4) as osbp, \
             tc.tile_pool(name="ops", bufs=4, space="PSUM") as ops, \
             tc.tile_pool(name="gps", bufs=4, space="PSUM") as gps:
            for ho in range(NH):
                wo_sb = wost.tile([P, H], BF16, name="wo_sb")
                nc.sync.dma_start(out=wo_sb, in_=woT[ho])
                wg_sb = wost.tile([P, H], BF16, name="wg_sb")
                nc.sync.dma_start(out=wg_sb, in_=wgT[ho])
                for lq in range(LQ):
                    opsum = ops.tile([P, 512], F32, name="opsum")
                    gpsum = gps.tile([P, 512], F32, name="gpsum")
                    for kt in range(KT):
                        nc.tensor.matmul(
                            opsum, wo_sb[:, kt * P:(kt + 1) * P],
                            qa_big[:, kt, lq * 512:(lq + 1) * 512],
                            start=(kt == 0), stop=(kt == KT - 1))
                        nc.tensor.matmul(
                            gpsum, wg_sb[:, kt * P:(kt + 1) * P],
                            x_big[:, kt, lq * 512:(lq + 1) * 512],
                            start=(kt == 0), stop=(kt == KT - 1))
                    # gate = sigmoid(gpsum * 1/rms); out = gate * opsum
                    g_sb = gsbp.tile([P, 512], F32, name="g_sb")
                    nc.vector.tensor_mul(
                        g_sb, gpsum, s_bc[:, lq * 512:(lq + 1) * 512])
                    g_sg = gsbp.tile([P, 512], BF16, name="g_sg")
                    nc.scalar.activation(
                        g_sg, g_sb, mybir.ActivationFunctionType.Sigmoid)
                    o_sb = osbp.tile([P, 512], BF16, name="o_sb")
                    nc.vector.tensor_mul(o_sb, opsum, g_sg)
                    nc.sync.dma_start(
                        out=outT[ho * P:(ho + 1) * P,
                                 lq * 512:(lq + 1) * 512],
                        in_=o_sb)

    nc.compile()
    return nc


import numpy as np
import ml_dtypes

BF_NP = ml_dtypes.bfloat16

_H, _NH, _HD, _M = 2048, 16, 128, 256
_B, _L = 4, 4096
_RPC = 2048          # rows per core
_NCORES = 8
_EPS = 1e-6

_nc_cache = [None]


def _pack_proj(wT):
    """[H, H] lhsT (in, out) -> [NH, 128, H] per-head-column staging:
    arr[ho, p, kt*128 + c] = wT[kt*128 + p, ho*128 + c]."""
    KT = wT.shape[0] // 128
    NH = wT.shape[1] // 128
    t = wT.reshape(KT, 128, NH, 128)            # kt, p, ho, c
    return np.ascontiguousarray(t.transpose(2, 1, 0, 3).reshape(NH, 128, KT * 128))


def _tile_w(wT, width):
    KT = wT.shape[0] // 128
    n = wT.shape[1] // width
    return np.ascontiguousarray(
        wT.reshape(KT, 128, n, width).transpose(2, 0, 1, 3))


def kernel(hidden_states, memory_tokens, memory_mask, norm_w,
           wq, wk, wv, wo, wg):
    import concourse.bacc as bacc

    hs = np.asarray(hidden_states, dtype=np.float32)
    mem = np.asarray(memory_tokens, dtype=np.float32)
    mask = np.asarray(memory_mask)
    norm_w = np.asarray(norm_w, dtype=np.float32)

    wq_n = (np.asarray(wq, dtype=np.float32) * norm_w[None, :]).T
    wg_n = (np.asarray(wg, dtype=np.float32) * norm_w[None, :]).T
    shared = {
        "wqT": _pack_proj(wq_n).astype(BF_NP),
        "wgT": _pack_proj(wg_n).astype(BF_NP),
        "woT": _pack_proj(np.asarray(wo, dtype=np.float32).T).astype(BF_NP),
        "wkT": _tile_w(np.asarray(wk, dtype=np.float32).T, 1024).astype(BF_NP),
        "wvT": _tile_w(np.asarray(wv, dtype=np.float32).T, 512).astype(BF_NP),
    }

    in_maps = []
    for c in range(_NCORES):
        b, half = c // 2, c % 2
        inp = dict(shared)
        inp["xT"] = np.ascontiguousarray(
            hs[b, half * _RPC:(half + 1) * _RPC, :].T).astype(BF_NP)
        inp["memT"] = np.ascontiguousarray(mem[b].T).astype(BF_NP)
        maskb = np.where(mask[b], 0.0, -50.0).astype(np.float32)
        inp["maskb"] = np.ascontiguousarray(maskb.reshape(_M // 128, 128).T)
        in_maps.append(inp)

    if _nc_cache[0] is None:
        nc = bacc.Bacc(None, target_bir_lowering=False, debug=False)
        build(nc, _H, _NH, _RPC, _M, eps=_EPS)
        _nc_cache[0] = nc
    nc = _nc_cache[0]

    import os
    trace = os.environ.get("KERNEL_TRACE") == "1"
    res = run_bass_kernel_spmd(nc, in_maps, core_ids=list(range(_NCORES)),
                               trace=trace)
    kernel.last_result = res

    out = np.empty((_B, _L, _H), dtype=np.float32)
    for c in range(_NCORES):
        b, half = c // 2, c % 2
        out[b, half * _RPC:(half + 1) * _RPC, :] = \
            res.results[c]["outT"].T.astype(np.float32)
    return out
